# revision 2
# baseline (speedup 1.0000x reference)
"""Trainium2 Bass kernel for nn_BRGEHHNet (gnn_message_passing).

Contract: kernel(**inputs) takes FULL unsharded inputs (as produced by
setup_inputs) and returns the FULL (25, 2048) float32 output.

Strategy: data-parallel over the batch dim across 8 NeuronCores.
Each core handles a 256-column batch shard; the small anova/adjacency
and per-agent critic weights are replicated. BatchNorm statistics are
over the full batch, so every core loads the full transposed states
(6.5 MB) and computes the stats locally (no collectives).

Matmuls run in bf16 (fp32 PSUM accumulation); batch-norm statistics and
the attention fold stay fp32.

Math notes:
  - att_out = emb * all_att broadcast followed by per-agent critics is
    folded into the critic weights: h1 = emb @ (all_att.T expanded * w1T).
  - The adjacency scatter (last-write-wins, matching jax-CPU/torch
    semantics) is encoded host-side as a 0/1 selection matrix S_sel
    (150 x 3200, uint8); neighbor_att = S_sel[:, r].T @ bi_att on the PE.
  - w2/w3 per-agent critics become block-diagonal matmuls; biases are
    folded into the PSUM accumulation as K=1 matmuls against a ones row.
  - The action gather is a one-hot mask multiply on the vector engine.
"""

import os
import numpy as np

import concourse.bacc as bacc
import concourse.mybir as mybir
import concourse.tile as tile
from concourse.tile import add_dep_helper
from concourse import bass_utils

N_CORES = 8
A = 25          # agents
B = 2048        # batch
S = 32          # state dim
F = A * S       # 800 features (contraction of M1)
E = 3200        # EHH_HID (= 25 * 128)
R = A * 12      # 300 critic hidden rows
INTER = 150
NA = 4
BSH = B // N_CORES  # 256 per-core batch shard

F_T = [128] * 6 + [32]          # feature tiles (800 = 6*128 + 32)
E_MT = E // 128                  # 25 output tiles of M1
R_SPLIT = [(0, 128), (128, 256), (256, 300)]   # (a,k) row tiling
WCH = 1024                       # ehh_w column-chunk width (8 mt per chunk)
W1CH = 5                         # w1T chunk: 5 mt row-tiles per DMA

DT = mybir.dt
F32 = DT.float32
BF16 = DT.bfloat16
I32 = DT.int32
U8 = DT.uint8

TRACE = os.environ.get("BASS_KERNEL_TRACE", "0") == "1"
LAST_EXEC_NS = None

_CACHE = {}


def _build_program():
    nc = bacc.Bacc("TRN2", target_bir_lowering=False, debug=False,
                   num_devices=N_CORES)

    sT_d = nc.dram_tensor("sT", [F, B], F32, kind="ExternalInput")
    ehh_w_d = nc.dram_tensor("ehh_w", [F, E], F32, kind="ExternalInput")
    w1T_d = nc.dram_tensor("w1T", [E, R], F32, kind="ExternalInput")
    ssel_d = nc.dram_tensor("ssel", [INTER, E], U8, kind="ExternalInput")
    aself_d = nc.dram_tensor("aself", [E, A], F32, kind="ExternalInput")
    bi_d = nc.dram_tensor("bi", [INTER, A], F32, kind="ExternalInput")
    bd2_d = nc.dram_tensor("bd2", [R, R], F32, kind="ExternalInput")
    bd3_d = nc.dram_tensor("bd3", [R, 128], F32, kind="ExternalInput")
    b1_d = nc.dram_tensor("b1r", [1, R], F32, kind="ExternalInput")
    b2_d = nc.dram_tensor("b2r", [1, R], F32, kind="ExternalInput")
    b3_d = nc.dram_tensor("b3r", [1, 128], F32, kind="ExternalInput")
    ones_d = nc.dram_tensor("ones", [1, BSH], F32, kind="ExternalInput")
    act_d = nc.dram_tensor("act", [A, BSH], I32, kind="ExternalInput")
    out_d = nc.dram_tensor("out", [A, BSH], F32, kind="ExternalOutput")

    with tile.TileContext(nc) as tc:
        with (
            tc.tile_pool(name="const", bufs=1) as cpool,
            tc.tile_pool(name="xt", bufs=7) as xt_pool,
            tc.tile_pool(name="xn", bufs=7) as xn_pool,
            tc.tile_pool(name="st", bufs=7) as st_pool,
            tc.tile_pool(name="big", bufs=1) as big_pool,
            tc.tile_pool(name="w1e", bufs=25) as w1e_pool,
            tc.tile_pool(name="spn", bufs=5) as spn_pool,
            tc.tile_pool(name="emb", bufs=8) as emb_pool,
            tc.tile_pool(name="hh", bufs=8) as h_pool,
            tc.tile_pool(name="ps", bufs=4, space="PSUM") as ps_pool,
            tc.tile_pool(name="psatt", bufs=1, space="PSUM") as psatt_pool,
            tc.tile_pool(name="psh1", bufs=3, space="PSUM") as psh1_pool,
        ):
            # ---- load x^T tiles, batch-norm stats, normalize own shard ----
            xn = []
            for k in range(7):
                rows = F_T[k]
                r0 = k * 128
                xt = xt_pool.tile([128, B], F32, tag="xt")
                nc.gpsimd.dma_start(xt[0:rows, :], sT_d.ap()[r0:r0 + rows, :])
                ssum = st_pool.tile([128, 4], F32, tag="st")
                bnst = st_pool.tile([128, 24], F32, tag="bnst")
                # per-feature mean/var over the full batch (4 chunked passes)
                for g in range(4):
                    nc.vector.bn_stats(
                        bnst[0:rows, 6 * g:6 * g + 6],
                        xt[0:rows, 512 * g:512 * (g + 1)])
                nc.vector.bn_aggr(ssum[0:rows, 0:2], bnst[0:rows, :])
                nc.vector.tensor_scalar(
                    ssum[0:rows, 1:2], ssum[0:rows, 1:2], 1e-5, None,
                    op0=mybir.AluOpType.add)
                nc.scalar.activation(
                    ssum[0:rows, 2:3], ssum[0:rows, 1:2],
                    mybir.ActivationFunctionType.Sqrt)
                nc.vector.reciprocal(ssum[0:rows, 3:4], ssum[0:rows, 2:3])
                xnk = xn_pool.tile([128, BSH], BF16, tag="xn")
                nc.vector.tensor_scalar(
                    xnk[0:rows, :], xt[0:rows, 0:BSH],
                    ssum[0:rows, 0:1], ssum[0:rows, 3:4],
                    op0=mybir.AluOpType.subtract, op1=mybir.AluOpType.mult)
                xn.append(xnk)

            # ---- constant / small inputs; explicit SWDGE issue order ----
            ssel0 = cpool.tile([128, E], BF16, tag="ssel0")
            ssel1 = cpool.tile([INTER - 128, E], BF16, tag="ssel1")
            nc.gpsimd.dma_start(ssel0[:], ssel_d.ap()[0:128, :])
            nc.gpsimd.dma_start(ssel1[:], ssel_d.ap()[128:INTER, :])
            bi0 = cpool.tile([128, A], BF16, tag="bi0")
            bi1 = cpool.tile([INTER - 128, A], BF16, tag="bi1")
            nc.gpsimd.dma_start(bi0[:], bi_d.ap()[0:128, :])
            nc.gpsimd.dma_start(bi1[:], bi_d.ap()[128:INTER, :])

            # big weights: bf16/f32 in SBUF
            wfull = big_pool.tile([128, 7 * E], BF16, tag="wfull")
            w1full = big_pool.tile([128, E_MT * R], F32, tag="w1full")
            spn_g = []
            for g in range(5):
                t = spn_pool.tile([128, 125], F32, tag="spn", name=f"spn_{g}")
                spn_g.append(t)

            # interleaved weight streams, gated behind most of the xT stream
            for g in range(4):
                g0 = g * WCH
                g1 = min(E, g0 + WCH)
                wd1 = nc.gpsimd.dma_start(
                    wfull[:].rearrange("p (k c) -> p k c", c=E)[:, 0:6, g0:g1],
                    ehh_w_d.ap()[0:768, g0:g1]
                    .rearrange("(k p) c -> p k c", p=128))

                nc.gpsimd.dma_start(
                    wfull[0:32, 6 * E + g0:6 * E + g1],
                    ehh_w_d.ap()[768:800, g0:g1])
                if g < 5:
                    m1 = min(E_MT, g * W1CH + W1CH)
                    nc.gpsimd.dma_start(
                        w1full[:].rearrange("p (m c) -> p m c", c=R)
                        [:, g * W1CH:m1, :],
                        w1T_d.ap()[g * W1CH * 128:m1 * 128, :]
                        .rearrange("(m p) c -> p m c", p=128))
                    nmt = min(5, E_MT - g * 5)
                    nc.gpsimd.dma_start(
                        spn_g[g][:, 0:nmt * A].rearrange("p (m c) -> p m c", c=A),
                        aself_d.ap()[g * 5 * 128:(g * 5 + nmt) * 128, :]
                        .rearrange("(m p) c -> p m c", p=128))
            for g in (4,):
                m1 = E_MT
                nc.gpsimd.dma_start(
                    w1full[:].rearrange("p (m c) -> p m c", c=R)
                    [:, g * W1CH:m1, :],
                    w1T_d.ap()[g * W1CH * 128:m1 * 128, :]
                    .rearrange("(m p) c -> p m c", p=128))
                nmt = min(5, E_MT - g * 5)
                nc.gpsimd.dma_start(
                    spn_g[g][:, 0:nmt * A].rearrange("p (m c) -> p m c", c=A),
                    aself_d.ap()[g * 5 * 128:(g * 5 + nmt) * 128, :]
                    .rearrange("(m p) c -> p m c", p=128))

            ones_t = cpool.tile([1, BSH], BF16, tag="ones")
            nc.gpsimd.dma_start(ones_t[:], ones_d.ap())
            b1_t = cpool.tile([1, R], BF16, tag="b1")
            b2_t = cpool.tile([1, R], BF16, tag="b2")
            b3_t = cpool.tile([1, 128], BF16, tag="b3")
            nc.gpsimd.dma_start(b1_t[:], b1_d.ap())
            nc.gpsimd.dma_start(b2_t[:], b2_d.ap())
            nc.gpsimd.dma_start(b3_t[:], b3_d.ap())
            bd2_t = []
            for j, (c0, c1) in enumerate(R_SPLIT):
                t = cpool.tile([c1 - c0, R], BF16, tag=f"bd2_{j}",
                               name=f"bd2t_{j}")
                nc.gpsimd.dma_start(t[:], bd2_d.ap()[c0:c1, :])
                bd2_t.append(t)
            bd3_t = []
            for j, (c0, c1) in enumerate(R_SPLIT):
                t = cpool.tile([c1 - c0, 128], BF16, tag=f"bd3_{j}",
                               name=f"bd3t_{j}")
                nc.gpsimd.dma_start(t[:], bd3_d.ap()[c0:c1, :])
                bd3_t.append(t)
            act_i = cpool.tile([A, BSH], I32, tag="acti")
            nc.sync.dma_start(act_i[:], act_d.ap())
            act_f = cpool.tile([A, BSH], F32, tag="actf")
            nc.vector.tensor_copy(act_f[:], act_i[:])
            masks = []
            for c4 in range(NA):
                mask = cpool.tile([A, BSH], F32, tag=f"mask_{c4}",
                                  name=f"mask_{c4}")
                nc.vector.tensor_scalar(
                    mask[:], act_f[:], float(c4), None,
                    op0=mybir.AluOpType.is_equal)
                masks.append(mask)

            # ---- attention + W1eff precompute (phase B) ----
            w1e_all = []
            for g in range(5):
                nmt = min(5, E_MT - g * 5)
                ps_att = psatt_pool.tile([128, 125], F32, tag="psatt",
                                         name=f"psatt_{g}")
                spn = spn_g[g]
                for l in range(nmt):
                    mt = g * 5 + l
                    sl = ps_att[:, l * A:(l + 1) * A]
                    nc.tensor.matmul(sl, ssel0[:, mt * 128:(mt + 1) * 128],
                                     bi0[:], start=True, stop=False)
                    nc.tensor.matmul(sl, ssel1[:, mt * 128:(mt + 1) * 128],
                                     bi1[:], start=False, stop=True)
                for l in range(nmt):
                    mt = g * 5 + l
                    sl = ps_att[:, l * A:(l + 1) * A]
                    spn_sl = spn[:, l * A:(l + 1) * A]
                    nc.vector.tensor_tensor(out=spn_sl, in0=spn_sl, in1=sl,
                                            op=mybir.AluOpType.add)
                    w1t = w1full[:, mt * R:(mt + 1) * R]
                    w1e_t = w1e_pool.tile([128, R], BF16, tag="w1e",
                                          name=f"w1e_{mt}")
                    nc.vector.tensor_tensor(
                        out=w1e_t[:].rearrange("p (a k) -> p a k", k=12),
                        in0=w1t.rearrange("p (a k) -> p a k", k=12),
                        in1=spn_sl.unsqueeze(2).broadcast_to((128, A, 12)),
                        op=mybir.AluOpType.mult)
                    w1e_all.append(w1e_t)

            # ---- main loop: M1, with M2 software-pipelined 2 tiles behind ----
            h1ps = [psh1_pool.tile([128, BSH], F32, tag="h1ps", name=f"h1ps_{j}")
                    for j in range(3)]
            embs = []

            def emit_m2(mt):
                for j, (c0, c1) in enumerate(R_SPLIT):
                    nc.tensor.matmul(h1ps[j][0:c1 - c0, :],
                                     w1e_all[mt][:, c0:c1], embs[mt][:],
                                     start=(mt == 0), stop=False)

            for mt in range(E_MT):
                # M1: emb^T tile = leaky(ehh_w^T @ xn)
                ps_mt = ps_pool.tile([128, BSH], F32, tag="ps", name=f"psm_{mt}")
                for k in range(7):
                    rows = F_T[k]
                    lhsT = wfull[0:rows, k * E + mt * 128:k * E + (mt + 1) * 128]
                    nc.tensor.matmul(ps_mt[:], lhsT, xn[k][0:rows, :],
                                     start=(k == 0), stop=(k == 6))
                emb = emb_pool.tile([128, BSH], BF16, tag="emb")
                nc.scalar.activation(emb[:], ps_mt[:],
                                     mybir.ActivationFunctionType.Lrelu,
                                     alpha=0.01)
                embs.append(emb)
                if mt >= 2:
                    emit_m2(mt - 2)
            for t in range(2, 0, -1):
                emit_m2(E_MT - t)

            # finish M2: bias row, then leaky
            h1 = []
            for j, (c0, c1) in enumerate(R_SPLIT):
                w = c1 - c0
                nc.tensor.matmul(h1ps[j][0:w, :], b1_t[:, c0:c1], ones_t[:],
                                 start=False, stop=True)
                t = h_pool.tile([128, BSH], BF16, tag=f"h1_{j}", name=f"h1_{j}")
                nc.scalar.activation(t[0:w, :], h1ps[j][0:w, :],
                                     mybir.ActivationFunctionType.Lrelu,
                                     alpha=0.01)
                h1.append(t)

            # M3: h2 = leaky(BD2^T @ h1 + b2)
            h2 = []
            for j, (c0, c1) in enumerate(R_SPLIT):
                w = c1 - c0
                ps3 = ps_pool.tile([128, BSH], F32, tag="ps", name=f"ps3_{j}")
                for k3, (k0, k1) in enumerate(R_SPLIT):
                    nc.tensor.matmul(ps3[0:w, :], bd2_t[k3][:, c0:c1],
                                     h1[k3][0:k1 - k0, :],
                                     start=(k3 == 0), stop=False)
                nc.tensor.matmul(ps3[0:w, :], b2_t[:, c0:c1], ones_t[:],
                                 start=False, stop=True)
                t = h_pool.tile([128, BSH], BF16, tag=f"h2_{j}", name=f"h2_{j}")
                nc.scalar.activation(t[0:w, :], ps3[0:w, :],
                                     mybir.ActivationFunctionType.Lrelu,
                                     alpha=0.01)
                h2.append(t)

            # M4: all_q^T (rows = c*32+a) = BD3^T @ h2 + b3
            ps_q = ps_pool.tile([128, BSH], F32, tag="ps", name="psq")
            for k4, (k0, k1) in enumerate(R_SPLIT):
                nc.tensor.matmul(ps_q[:], bd3_t[k4][:, :], h2[k4][0:k1 - k0, :],
                                 start=(k4 == 0), stop=False)
            nc.tensor.matmul(ps_q[:], b3_t[:], ones_t[:], start=False, stop=True)

            # gather: q[a, b] = all_q[c(a,b)*32+a, b] via one-hot masks
            qs = []
            for c4 in range(NA):
                qc = cpool.tile([A, BSH], F32, tag=f"qc_{c4}", name=f"qc_{c4}")
                nc.vector.tensor_tensor(
                    out=qc[:], in0=ps_q[c4 * 32:c4 * 32 + A, :], in1=masks[c4][:],
                    op=mybir.AluOpType.mult)
                qs.append(qc)
            nc.vector.tensor_tensor(out=qs[0][:], in0=qs[0][:], in1=qs[1][:],
                                    op=mybir.AluOpType.add)
            nc.vector.tensor_tensor(out=qs[2][:], in0=qs[2][:], in1=qs[3][:],
                                    op=mybir.AluOpType.add)
            nc.vector.tensor_tensor(out=qs[0][:], in0=qs[0][:], in1=qs[2][:],
                                    op=mybir.AluOpType.add)
            nc.sync.dma_start(out_d.ap(), qs[0][:])

    nc.compile()
    return nc


def _host_prep(inputs):
    states = np.asarray(inputs["states"], dtype=np.float32)
    ehh_w = np.ascontiguousarray(np.asarray(inputs["ehh_w"], dtype=np.float32))
    anova = np.asarray(inputs["anova"], dtype=np.float32)
    w1 = np.asarray(inputs["w1"], dtype=np.float32)
    b1 = np.asarray(inputs["b1"], dtype=np.float32)
    w2 = np.asarray(inputs["w2"], dtype=np.float32)
    b2 = np.asarray(inputs["b2"], dtype=np.float32)
    w3 = np.asarray(inputs["w3"], dtype=np.float32)
    b3 = np.asarray(inputs["b3"], dtype=np.float32)
    actions = np.asarray(inputs["actions"], dtype=np.int32)
    adj = np.asarray(inputs["adj"], dtype=np.int64)

    sT = np.ascontiguousarray(states.transpose(0, 2, 1).reshape(F, B))
    w1T = np.ascontiguousarray(w1.transpose(1, 0, 2).reshape(E, R))

    # adjacency scatter -> winning source row per target (last write wins,
    # col-3 scatter applied after col-1 scatter)
    src = np.full(E, -1, dtype=np.int64)
    for e in range(adj.shape[0]):
        src[adj[e, 1]] = adj[e, 0]
    for e in range(adj.shape[0]):
        src[adj[e, 3]] = adj[e, 0]
    ssel = np.zeros((INTER, E), dtype=np.uint8)
    hit = np.nonzero(src >= 0)[0]
    ssel[src[hit], hit] = 1

    bd2 = np.zeros((R, R), dtype=np.float32)
    bd3 = np.zeros((R, 128), dtype=np.float32)
    b3r = np.zeros((1, 128), dtype=np.float32)
    for a in range(A):
        bd2[12 * a:12 * a + 12, 12 * a:12 * a + 12] = w2[a]
        for c in range(NA):
            bd3[12 * a:12 * a + 12, c * 32 + a] = w3[a, :, c]
            b3r[0, c * 32 + a] = b3[a, c]

    common = {
        "ehh_w": ehh_w,
        "w1T": w1T,
        "ssel": ssel,
        "aself": np.ascontiguousarray(anova[:E]),
        "bi": np.ascontiguousarray(anova[E:]),
        "bd2": bd2,
        "bd3": bd3,
        "b1r": b1.reshape(1, R).copy(),
        "b2r": b2.reshape(1, R).copy(),
        "b3r": b3r,
        "ones": np.ones((1, BSH), dtype=np.float32),
    }
    in_maps = []
    for c in range(N_CORES):
        m = dict(common)
        m["sT"] = np.ascontiguousarray(np.roll(sT, -BSH * c, axis=1))
        m["act"] = np.ascontiguousarray(actions[:, BSH * c:BSH * (c + 1)])
        in_maps.append(m)
    return in_maps


def kernel(**inputs):
    global LAST_EXEC_NS, LAST_RES
    if "nc" not in _CACHE:
        _CACHE["nc"] = _build_program()
    nc = _CACHE["nc"]
    in_maps = _host_prep(inputs)
    kwargs = {}
    if TRACE:
        import shutil
        shutil.rmtree("/tmp/bass_trace", ignore_errors=True)
        os.makedirs("/tmp/bass_trace", exist_ok=True)
        kwargs["trace"] = True
        kwargs["tmpdir"] = "/tmp/bass_trace"
    res = bass_utils.run_bass_kernel_spmd(
        nc, in_maps, core_ids=list(range(N_CORES)), **kwargs)
    LAST_RES = res
    LAST_EXEC_NS = res.exec_time_ns
    q = np.empty((A, B), dtype=np.float32)
    for c in range(N_CORES):
        q[:, BSH * c:BSH * (c + 1)] = res.results[c]["out"]
    return q

